# revision 34
# baseline (speedup 1.0000x reference)
"""EquiAttention Trainium2 kernel.

Computes the reference nn_EquiAttention forward pass on 8 NeuronCores,
data-parallel over the batch axis (64 batches -> 8 per core).

Math refactoring (validated exact in float64):
  The reference builds q/k embeddings of width 192:
    q = [ (Wq @ vecs).flat (128) , scalars @ Wq_s.T + bq_s (64) ]
    k = [ (Wk @ vecs * METRIC).flat (128) , scalars @ Wk_s.T + bk_s (64) ]
  Softmax over keys is invariant to per-query constants, so scores fold
  to a 128-dim contraction plus a per-key bias:
    scores[q,m] ~ qv_q.kv_m + s_q.(H s_m) + c2.s_m    (mod per-q const)
  with  qv = vecs.flat (64),  kv[(j,k),m] = scale*METRIC[k]*(G @ vecs[..,k])
        G = Wq.T @ Wk,  H = scale * Wq_s.T @ Wk_s,  c2 = scale * Wk_s.T @ bq_s
  The per-key bias folds into V:  w_m = exp(c2.s_m) (values ~[0.6,1.6]),
  Vaug[m] = [w_m * v_m, w_m];  out = acc[:, :64] / acc[:, 64].

Device structure per batch (per core):
  - qT [128,N] = [vecsT ; scalarsT] via PE transposes of the combined
    normalized-vector/scalar chunks; kT = blockdiag(G~,H~).T @ qT.
    Both are split hi/lo into fp16 pairs; 3-pass scores
    (qhi.khi + qhi.klo + qlo.khi) are exact to ~1e-4 absolute because
    max_row sum|q||k| ~ 117 (no catastrophic cancellation), so the PE
    runs at 1 cycle/row instead of fp32's 4.
  - scores per 128-query block land in two half-bank PSUM tiles
    (4-deep pool -> deep matmul/softmax pipelining); row-max via DVE
    reduce per half + min; P = exp(S-max) written fp16 by ACT.
  - P^T via DMA xbar transpose (one DMA per query block; all xbar
    transposes kept on one HWDGE queue -- concurrent xbar use from two
    queues returned corrupt data on HW) into ptf[p, qb, mc, q'] so each
    transpose destination is contiguous.
  - P @ Vaug computed transposed per query-half: accT[65, 512] +=
    Vaug[mc].T @ P^T chunks (fp16, 512-wide moving), PE-transposed back
    per query block, normalized by the denominator column, and written
    out with one DMA per half.
"""

import numpy as np

B, N = 64, 1024
NCORES = 8
BL = B // NCORES          # batches per core
NB = N // 128             # 128-row blocks per sequence
SCALE = 1.0 / np.sqrt(192.0)

_CACHE = {}


def _build_program():
    import concourse.bacc as bacc
    import concourse.tile as tile
    from concourse import mybir

    f32 = mybir.dt.float32

    nc = bacc.Bacc("TRN2", target_bir_lowering=False,
                   debug=False, num_devices=NCORES)

    aps = {
        "vectors": nc.dram_tensor("vectors", [BL, N, 64], f32,
                                  kind="ExternalInput").ap(),
        "scalars": nc.dram_tensor("scalars", [BL, N, 64], f32,
                                  kind="ExternalInput").ap(),
        "BD": nc.dram_tensor("BD", [128, 128], f32, kind="ExternalInput").ap(),
        "WvC2": nc.dram_tensor("WvC2", [128, 65], f32, kind="ExternalInput").ap(),
        "out": nc.dram_tensor("out", [BL, N, 64], f32, kind="ExternalOutput").ap(),
    }

    with tile.TileContext(nc) as tc:
        _emit(tc, aps)

    nc.compile()
    return nc


def _emit(tc, aps):
    from contextlib import ExitStack
    import concourse.bass as bass
    import concourse.masks as masks
    from concourse import mybir

    nc = tc.nc
    f32 = mybir.dt.float32
    f16 = mybir.dt.float16
    PS = "PSUM"
    Act = mybir.ActivationFunctionType
    Alu = mybir.AluOpType
    X = mybir.AxisListType.X

    vecs_d, scal_d = aps["vectors"], aps["scalars"]
    bd_d, wvc2_d, out_d = aps["BD"], aps["WvC2"], aps["out"]

    with ExitStack() as ctx:
        singles = ctx.enter_context(tc.tile_pool(name="singles", bufs=1))
        raw = ctx.enter_context(tc.tile_pool(name="raw", bufs=2))
        emb = ctx.enter_context(tc.tile_pool(name="emb", bufs=2))
        small = ctx.enter_context(tc.tile_pool(name="small", bufs=6))
        pP = ctx.enter_context(tc.tile_pool(name="pP", bufs=4))
        pPT = ctx.enter_context(tc.tile_pool(name="pPT", bufs=2))
        outp = ctx.enter_context(tc.tile_pool(name="outp", bufs=4))
        accsb = ctx.enter_context(tc.tile_pool(name="accsb", bufs=4))
        psS = ctx.enter_context(tc.tile_pool(name="psS", bufs=4, space=PS))
        psAcc = ctx.enter_context(tc.tile_pool(name="psAcc", bufs=2, space=PS))
        psMisc = ctx.enter_context(tc.tile_pool(name="psMisc", bufs=2, space=PS))

        ident = singles.tile([128, 128], f32)
        masks.make_identity(nc, ident[:])
        bd = singles.tile([128, 128], f32)
        nc.gpsimd.dma_start(out=bd[:], in_=bd_d[:, :])
        wvc2_16 = singles.tile([128, 65], f16)
        nc.gpsimd.dma_start(out=wvc2_16[:], in_=wvc2_d[:, :].bitcast(f32))

        def embed_batch(b):
            # ---------------- embedding phase ----------------
            # combined [vec | scalar] chunk tile so one PE transpose per
            # chunk yields a full 128-row column block of qT
            vs = raw.tile([128, NB, 128], f32, tag="vs")
            nc.gpsimd.dma_start(out=vs[:, :, 64:128],
                                in_=scal_d[b].rearrange("(c p) f -> p c f", p=128))
            vraw = raw.tile([128, NB, 64], f32, tag="vraw")
            nc.gpsimd.dma_start(out=vraw[:],
                                in_=vecs_d[b].rearrange("(c p) f -> p c f", p=128))

            # Lorentz normalization of the 16 four-vectors per particle
            sq = raw.tile([128, NB, 16, 4], f32, tag="sq")
            nc.scalar.activation(out=sq[:], in_=vraw[:], func=Act.Square)
            nrm = raw.tile([128, NB, 16], f32, tag="nrm")
            nc.vector.tensor_sub(nrm[:], sq[:, :, :, 0], sq[:, :, :, 1])
            nc.vector.tensor_sub(nrm[:], nrm[:], sq[:, :, :, 2])
            nc.vector.tensor_sub(nrm[:], nrm[:], sq[:, :, :, 3])
            nc.scalar.activation(out=nrm[:], in_=nrm[:], func=Act.Abs)
            nc.vector.tensor_scalar_max(nrm[:], nrm[:], 1e-5)
            nc.scalar.activation(out=nrm[:], in_=nrm[:], func=Act.Sqrt)
            rn = raw.tile([128, NB, 16], f32, tag="rn")
            nc.vector.reciprocal(rn[:], nrm[:])

            qT = emb.tile([128, N], f32, tag="qT")
            qhi = emb.tile([128, N], f16, tag="qhi")
            qlo = emb.tile([128, N], f16, tag="qlo")
            khi = emb.tile([128, N], f16, tag="khi")
            klo = emb.tile([128, N], f16, tag="klo")
            half = NB // 2
            for hh in range(2):
                cs = slice(hh * half, (hh + 1) * half)
                rn_b = bass.AP(tensor=rn.tensor,
                               offset=rn.offset + hh * half * rn.ap[1][0],
                               ap=[rn.ap[0], [rn.ap[1][0], half], rn.ap[2],
                                   [0, 4]])
                nc.vector.tensor_mul(
                    vs[:, cs, 0:64].rearrange("p c (j k) -> p c j k", k=4),
                    vraw[:, cs].rearrange("p c (j k) -> p c j k", k=4), rn_b)
                # four transposes into one PSUM bank, then one copy
                pt = psMisc.tile([128, 512], f32, tag="misc")
                for j, c in enumerate(range(hh * half, (hh + 1) * half)):
                    nc.tensor.transpose(pt[:, j * 128:(j + 1) * 128],
                                        vs[:, c], ident[:])
                nc.vector.tensor_copy(qT[:, hh * 512:(hh + 1) * 512], pt[:])
                # fp16 hi/lo split of qT; 3-pass scores
                # qhi.khi + qhi.klo + qlo.khi are exact to ~1e-4 (max_row
                # sum|q||k| ~ 117 -> no catastrophic cancellation)
                cols = slice(hh * 512, (hh + 1) * 512)
                nc.vector.tensor_copy(qhi[:, cols], qT[:, cols])
                nc.vector.tensor_sub(qlo[:, cols], qT[:, cols], qhi[:, cols])
                # kT = blockdiag(G~, H~).T @ qT, hi/lo split from PSUM
                pk = psMisc.tile([128, 512], f32, tag="misc")
                nc.tensor.matmul(pk[:], bd[:], qT[:, cols],
                                 start=True, stop=True)
                nc.scalar.copy(khi[:, cols], pk[:])
                nc.vector.tensor_sub(klo[:, cols], pk[:], khi[:, cols])

            # Vaug chunks (natural key order, matching the xbar block
            # transpose): Vaug[m] = [w_m * v_m, w_m], w = exp(c2.s)
            vaug = emb.tile([128, NB, 65], f16, tag="vaug")
            for mc in range(NB):
                csel = qhi[:, mc * 128:(mc + 1) * 128]
                pv = psMisc.tile([128, 65], f32, tag="misc")
                nc.tensor.matmul(pv[:], csel, wvc2_16[:], start=True, stop=True)
                nc.scalar.activation(out=vaug[:, mc, 64:65], in_=pv[:, 64:65],
                                     func=Act.Exp)
                wcol = small.tile([128, 1], f32, tag="wcol")
                nc.scalar.activation(out=wcol[:], in_=pv[:, 64:65], func=Act.Exp)
                nc.scalar.activation(out=vaug[:, mc, 0:64], in_=pv[:, 0:64],
                                     func=Act.Copy, scale=wcol[:])
            return qhi, qlo, khi, klo, vaug

        def attn_batch(b, emb_tiles):
            qhi, qlo, khi, klo, vaug = emb_tiles
            # ---------------- attention phase ----------------
            # P^T layout: ptf[p, qb, mc, q'] = P[qb*128+q', mc*128+p];
            # dims ordered so each DMA-transpose destination ptf[:, qb]
            # is contiguous per partition (sliced dst is wrong on HW)
            ptf = pPT.tile([128, NB, NB, 128], f16, tag="ptf")

            def q_block(qb):
                qs = slice(qb * 128, (qb + 1) * 128)
                Sh, m01 = [], []
                for h in range(2):
                    cols = slice(h * 512, (h + 1) * 512)
                    S = psS.tile([128, 512], f32, tag="S")
                    nc.tensor.matmul(S[:], qhi[:, qs], khi[:, cols],
                                     start=True, stop=False)
                    nc.tensor.matmul(S[:], qhi[:, qs], klo[:, cols],
                                     start=False, stop=False)
                    nc.tensor.matmul(S[:], qlo[:, qs], khi[:, cols],
                                     start=False, stop=True)
                    m = small.tile([128, 1], f32, tag="m01")
                    nc.vector.tensor_reduce(m[:], S[:], axis=X,
                                            op=Alu.max, negate=True)
                    Sh.append(S)
                    m01.append(m)
                negmax = small.tile([128, 1], f32, tag="negmax")
                nc.vector.tensor_tensor(negmax[:], m01[0][:], m01[1][:],
                                        op=Alu.min)
                P = pP.tile([128, N], f16, tag="P")
                for h in range(2):
                    nc.scalar.activation(out=P[:, h * 512:(h + 1) * 512],
                                         in_=Sh[h][:], func=Act.Exp,
                                         bias=negmax[:], scale=1.0)
                # xbar transpose: P [128q', 1024m] -> per-block P^T into
                # ptf[:, qb] (contiguous dst; keep all xbar transposes on
                # one queue -- concurrent xbar use from two HWDGE queues
                # returned corrupt data on HW)
                nc.sync.dma_start_transpose(ptf[:, qb], P[:])

            # accT[65, qhalf] += Vaug[mc].T @ P^T[mc] (fp16, 512-wide);
            # per-half acc banks so the epilogue starts mid-batch
            for qb in range(NB):
                q_block(qb)

            def pv_epi(hh):
                accT = psAcc.tile([65, 512], f32, tag="accT")
                for mc in range(NB):
                    nc.tensor.matmul(accT[:], vaug[:, mc, :],
                                     ptf[:, hh * 4:(hh + 1) * 4, mc, :],
                                     start=(mc == 0), stop=(mc == NB - 1))
                accsb_t = accsb.tile([65, 512], f32, tag="accsb")
                nc.vector.tensor_copy(accsb_t[:], accT[:])
                ot = psMisc.tile([128, 4, 65], f32, tag="misc")
                for j in range(4):
                    nc.tensor.transpose(ot[:, j], accsb_t[:, j * 128:(j + 1) * 128],
                                        ident[0:65, 0:65])
                rden = small.tile([128, 4], f32, tag="rden")
                nc.vector.reciprocal(rden[:], ot[:, :, 64])
                ob = outp.tile([128, 4, 64], f32, tag="ob")
                for j in range(4):
                    nc.vector.tensor_scalar_mul(ob[:, j], ot[:, j, 0:64],
                                                rden[:, j:j + 1])
                nc.gpsimd.dma_start(
                    out=out_d[b, hh * 512:(hh + 1) * 512, :]
                    .rearrange("(j p) f -> p j f", p=128),
                    in_=ob[:])

            pv_epi(0)
            pv_epi(1)

        prev = embed_batch(0)
        for b in range(1, BL):
            cur = embed_batch(b)
            attn_batch(b - 1, prev)
            prev = cur
        attn_batch(BL - 1, prev)


def _host_weights(Wq, Wk, Wv, Wq_s, Wk_s, bq_s):
    """Fold the tiny EquiLinear weights (float64 precompute, cast f32)."""
    METRIC = np.array([1.0, -1.0, -1.0, -1.0], dtype=np.float64)
    G = Wq.astype(np.float64).T @ Wk.astype(np.float64)            # [16,16]
    BD = np.zeros((128, 128), dtype=np.float64)
    for k in range(4):
        # lhsT[(j',k), (j,k)] = SCALE * METRIC[k] * G[j, j']
        BD[k:64:4, k:64:4] = SCALE * METRIC[k] * G.T
    # lhsT[h, g] = SCALE * H[g, h],  H = Wq_s.T @ Wk_s
    BD[64:, 64:] = SCALE * (Wk_s.astype(np.float64).T @ Wq_s.astype(np.float64))
    E = np.exp(Wv.astype(np.float64))                              # [16,16]
    WvC2 = np.zeros((128, 65), dtype=np.float64)
    for k in range(4):
        # rhs[(j,k), (i,k)] = E[i, j]
        WvC2[k:64:4, k:64:4] = E.T
    WvC2[64:, 64] = SCALE * (Wk_s.astype(np.float64).T @ bq_s.astype(np.float64))
    return (np.ascontiguousarray(BD, dtype=np.float32),
            np.ascontiguousarray(WvC2, dtype=np.float32))


def _prepare_in_maps(vectors, scalars, Wq, Wq_s, bq_s, Wk, Wk_s, bk_s, Wv):
    BD, WvC2 = _host_weights(Wq, Wk, Wv, Wq_s, Wk_s, bq_s)
    vecs_flat = np.ascontiguousarray(
        np.asarray(vectors).reshape(B, N, 64), dtype=np.float32)
    scal = np.ascontiguousarray(scalars, dtype=np.float32)

    in_maps = []
    for c in range(NCORES):
        sl = slice(c * BL, (c + 1) * BL)
        in_maps.append({
            "vectors": np.ascontiguousarray(vecs_flat[sl]),
            "scalars": np.ascontiguousarray(scal[sl]),
            "BD": BD,
            "WvC2": WvC2,
        })
    return in_maps


def _run(in_maps, **kw):
    from concourse.bass_utils import run_bass_kernel_spmd
    nc = _get_program()
    return run_bass_kernel_spmd(nc, in_maps, list(range(NCORES)), **kw)


def _get_program():
    if "nc" not in _CACHE:
        _CACHE["nc"] = _build_program()
    return _CACHE["nc"]


def kernel(vectors, scalars, Wq, Wq_s, bq_s, Wk, Wk_s, bk_s, Wv):
    args = [np.asarray(a, dtype=np.float32) for a in
            (vectors, scalars, Wq, Wq_s, bq_s, Wk, Wk_s, bk_s, Wv)]
    in_maps = _prepare_in_maps(*args)
    res = _run(in_maps)
    out = np.concatenate([res.results[c]["out"] for c in range(NCORES)], axis=0)
    return out.reshape(B, N, 16, 4).astype(np.float32)
